# revision 8
# baseline (speedup 1.0000x reference)
"""2-layer GCN (GCNConv x2 + ReLU) on 8 Trainium2 NeuronCores — bf16 edition.

Contract: kernel(**inputs) takes FULL inputs (x [100000,64] f32,
edge_index [2,1600000] i32, W1 [64,64], b1 [64], W2 [64,32], b2 [32])
and returns the FULL output [100000, 32] f32.

Strategy (graph/data parallel, hardcoded for these shapes):
  - GCN refactor: out = relu(dis * (scatter_dst(g[src]) + g[dst]) + b)
    with g = (act * dis) @ W.  dis = 1/sqrt(deg) is folded into the
    activations (host pre-scales x; the device rescales h1), so the dense
    phases are pure matmuls.
  - Nodes are assigned to 8 cores x 100 blocks of 128 dsts by a greedy
    capacity-constrained packer so nearly every per-(block, src-chunk)
    cell fits its 4*128 tile quota -> only a few % gather-slot padding.
  - All edge-phase operands are bf16: gather tables store 256B rows
    ([128 bf16] with 64/32 real feats), messages are dma_gather'd by
    int16 row index (4 chunk tables < 32767 rows each), scattered into
    PSUM via one-hot matmuls (lhsT = messages, rhs = one-hot).
  - One-hot built on DVE in a [slot, dst, col] layout where every operand
    is 2-byte packed (hits the 2x_1p DVE mode).
  - Self-loops never touch DMA: per-block identity matmuls add g[dst]
    from SBUF-resident own-shard tables.
  - The layer-2 tables are AllGather'd in compact [rows, 32] bf16 form
    (4 chunked collectives overlap the layer-1 edge phase), then expanded
    to 256B-stride rows by a strided DRAM-to-DRAM copy.
  - Both layers share one idx/dl staging (identical edge structure).
"""

import sys

if "/opt/trn_rl_repo" not in sys.path:
    sys.path.insert(0, "/opt/trn_rl_repo")

import numpy as np
import ml_dtypes

BF16 = ml_dtypes.bfloat16

N = 100000
IN = 64
HID = 64
OUT = 32
C = 8                  # cores
BLK = 128              # dst nodes per block / one-hot width
NBLK = 100             # blocks per core (12800 padded nodes)
NP = NBLK * BLK        # 12800 padded nodes per core
SWMAX = 9              # max blocks per sweep (3 PSUM banks at 64 parts)
DCH = 8                # dense-phase blocks per psum chunk (1 bank)
PADDL = 300.0          # dl for pad slots (no one-hot match)
OH_GRP = 8             # one-hot columns built per DVE instruction

QB = [25, 25, 25, 25]               # blocks per quarter (chunk)
QROWS = [b * BLK for b in QB]       # padded rows per (rank, chunk)
QBASE = np.cumsum([0] + QROWS[:-1])
TROWS = [C * r for r in QROWS]      # gather-table rows per chunk
assert max(TROWS) < 32767


def _sweeps():
    out = []
    for q, nq in enumerate(QB):
        left = nq
        while left > 0:
            take = min(SWMAX, left)
            out.append((take, q))
            left -= take
    return out


# ----------------------------------------------------------------------------
# Host-side packing
# ----------------------------------------------------------------------------

def _balance_assign(w, pool_sizes):
    """Capacity-constrained bin packing: per quarter, deal its nodes into
    C*QB[q] blocks of <=128 nodes so each per-(block, chunk) message count
    stays within the block's tile allocation (start at 4*128; bump a cell
    by one tile only when no block can absorb the node). Minimizes total
    tile quota = gather descriptors. Returns node->(core, padded offset)."""
    node_core = np.zeros(N, np.int32)
    node_off = np.zeros(N, np.int32)
    pb = np.cumsum([0] + pool_sizes)
    for q in range(4):
        ids = np.arange(pb[q], pb[q + 1])
        nb = C * QB[q]
        order = ids[np.argsort(-w[ids].sum(1), kind="stable")]
        sums = np.zeros((nb, 4), np.int64)
        caps = np.full((nb, 4), 4 * BLK, np.int64)
        cnt = np.zeros(nb, np.int64)
        gblk = np.zeros(order.size, np.int64)
        wv = w[order]
        for i in range(order.size):
            nxt = sums + wv[i]
            over = (nxt > caps).any(axis=1) | (cnt >= BLK)
            if not over.all():
                # spread: keep every cell's load low and even
                score = np.where(over, 1 << 60, nxt.max(axis=1) * 256 + cnt)
                b = int(np.argmin(score))
            else:
                # bump one cell's quota on the block needing least overflow
                excess = np.maximum(nxt - caps, 0).max(axis=1)
                excess[cnt >= BLK] = 1 << 60
                b = int(np.argmin(excess))
                caps[b] = np.maximum(caps[b], ((nxt[b] + BLK - 1) // BLK) * BLK)
            gblk[i] = b
            sums[b] += wv[i]
            cnt[b] += 1
        # refinement: relocate nodes out of overflowing cells
        local = {v: i2 for i2, v in enumerate(order)}
        for _ in range(6):
            overcells = np.argwhere(sums > 4 * BLK)
            if overcells.size == 0:
                break
            moved = 0
            for b, j in overcells:
                nodes_b = order[gblk == b]
                wb = w[nodes_b]
                cand = nodes_b[np.argsort(
                    -wb[:, j] + (wb[:, j] == 0) * (1 << 30), kind="stable")]
                for v in cand:
                    if sums[b, j] <= 4 * BLK or w[v, j] == 0:
                        break
                    nxt_all = sums + w[v]
                    ok = (~(nxt_all > 4 * BLK).any(axis=1)) & (cnt < BLK)
                    ok[b] = False
                    tb = np.flatnonzero(ok)
                    if tb.size == 0:
                        continue
                    t = int(tb[np.argmin(nxt_all[tb].max(axis=1))])
                    gblk[local[v]] = t
                    sums[b] -= w[v]
                    sums[t] += w[v]
                    cnt[b] -= 1
                    cnt[t] += 1
                    moved += 1
            if moved == 0:
                break
        # slot position within block
        pos = np.zeros(order.size, np.int64)
        srt = np.argsort(gblk, kind="stable")
        gs = gblk[srt]
        starts = np.searchsorted(gs, np.arange(nb))
        pos[srt] = np.arange(order.size) - starts[gs]
        core = gblk % C
        blk = QBASE[q] // BLK + gblk // C
        node_core[order] = core
        node_off[order] = blk * BLK + pos
    return node_core, node_off


def _pack(edge_index):
    src = np.asarray(edge_index[0], np.int64)
    dst = np.asarray(edge_index[1], np.int64)

    indeg = np.bincount(dst, minlength=N).astype(np.int64)
    deg = (indeg + 1).astype(np.float32)          # self-loop included
    dis = 1.0 / np.sqrt(deg)

    pool_sizes = [25000, 25000, 25000, N - 3 * 25000]
    pb = np.cumsum([0] + pool_sizes)
    srcq = (np.searchsorted(pb, src, side="right") - 1).astype(np.int64)
    w = np.zeros((N, 4), np.int64)
    for j in range(4):
        w[:, j] = np.bincount(dst[srcq == j], minlength=N)

    node_core, node_off = _balance_assign(w, pool_sizes)

    # src -> (chunk, table row)
    chunk = srcq                                   # == quarter of node_off
    off_s = node_off[src].astype(np.int64)
    assert (np.searchsorted(QBASE, off_s, side="right") - 1 == chunk).all()
    tidx = node_core[src] * np.asarray(QROWS)[chunk] + (off_s - QBASE[chunk])

    core = node_core[dst].astype(np.int64)
    dloc = node_off[dst].astype(np.int64)
    block = dloc // BLK
    dlb = dloc % BLK

    key = (core * NBLK + block) * 4 + chunk
    counts = np.bincount(key, minlength=C * NBLK * 4).reshape(C, NBLK, 4)
    quota = -(-counts.max(axis=0) // BLK)          # [NBLK, 4]

    sweeps = _sweeps()
    nsw = len(sweeps)
    szs = [s[0] for s in sweeps]
    sweep_base = np.cumsum([0] + szs[:-1])
    sweep_of_block = np.repeat(np.arange(nsw), szs)

    # global tile stream: for s, for j, for lb: quota tiles
    g_sj = np.zeros((nsw, 4), np.int64)
    for s in range(nsw):
        b0 = sweep_base[s]
        for j in range(4):
            g_sj[s, j] = quota[b0:b0 + szs[s], j].sum()
    call_base = np.zeros(nsw * 4, np.int64)
    np.cumsum(g_sj.reshape(-1)[:-1], out=call_base[1:])
    call_base = call_base.reshape(nsw, 4)
    tiles_total = int(g_sj.sum())
    slots_total = tiles_total * BLK

    # per-(block, chunk) tile base in the global stream
    cell_tbase = np.zeros((NBLK, 4), np.int64)
    for s in range(nsw):
        b0 = sweep_base[s]
        for j in range(4):
            cur = int(call_base[s, j])
            for lb in range(szs[s]):
                cell_tbase[b0 + lb, j] = cur
                cur += int(quota[b0 + lb, j])

    # schedule + start/stop flags per sweep
    # sequence: identity lb=0..nb-1, then (j, tiles in block order)
    sched = []           # sched[s][j] = [(cursor_in_call, lb, stop)]
    id_flags = []        # id_flags[s] = [(start, stop)] per lb
    for s in range(nsw):
        nb, _q = sweeps[s]
        b0 = sweep_base[s]
        nbank = (nb + 3) // 4
        last_touch = [("id", min(4 * k + 3, nb - 1)) for k in range(nbank)]
        seq = []
        for j in range(4):
            cur = 0
            call = []
            for lb in range(nb):
                for _r in range(int(quota[b0 + lb, j])):
                    call.append([cur, lb, False])
                    last_touch[lb // 4] = ("edge", j, len(call) - 1)
                    cur += 1
            seq.append(call)
        idf = [[lb % 4 == 0, False] for lb in range(nb)]
        for k in range(nbank):
            t = last_touch[k]
            if t[0] == "id":
                idf[t[1]][1] = True
            else:
                seq[t[1]][t[2]][2] = True
        sched.append(seq)
        id_flags.append(idf)

    meta = dict(quota=quota, sweeps=sweeps, sweep_base=sweep_base,
                g_sj=g_sj, call_base=call_base, tiles_total=tiles_total,
                slots_total=slots_total, sched=sched, id_flags=id_flags)

    # per-core slot fill
    per_core = []
    for c in range(C):
        m = core == c
        blk_c = block[m]
        ch_c = chunk[m]
        # slot = (cell_tbase[blk, ch]*128) + running index within cell
        cell_id = blk_c * 4 + ch_c
        order = np.argsort(cell_id, kind="stable")
        cid_s = cell_id[order]
        starts = np.searchsorted(cid_s, np.arange(NBLK * 4))
        pos = np.arange(cid_s.size) - starts[cid_s]
        slot = cell_tbase.reshape(-1)[cid_s] * BLK + pos
        assert (pos < quota.reshape(-1)[cid_s] * BLK).all()

        idx_slots = np.zeros(slots_total, np.int16)
        dl_slots = np.full(slots_total, PADDL, np.float32)
        idx_slots[slot] = tidx[m][order].astype(np.int16)
        dl_slots[slot] = dlb[m][order].astype(np.float32)

        idxw = np.tile(idx_slots.reshape(-1, 16).T.copy(), (8, 1))
        dlw = dl_slots.reshape(-1, BLK).T.astype(BF16).copy()

        # dis replicated across partitions, per padded node
        dis_own = np.ones(NP, np.float32)
        ids = np.where(node_core == c)[0]
        dis_own[node_off[ids]] = dis[ids]
        distT = np.tile(dis_own[None, :], (64, 1)).astype(BF16)

        per_core.append(dict(idxw=idxw, dlw=dlw, distT=distT))

    return meta, per_core, dis, node_core, node_off


def _stage_inputs(x, W1, b1, W2, b2, meta, per_core, dis, node_core, node_off):
    x = np.asarray(x, np.float32)
    xp = (x * dis[:, None]).astype(np.float32)     # fold dis[src] into x
    col = node_core.astype(np.int64) * NP + node_off
    xTf = np.zeros((IN, C * NP), np.float32)
    xTf[:, col] = xp.T
    xTf = xTf.astype(BF16)

    iota_rep = np.tile(np.repeat(np.arange(BLK, dtype=np.float32), OH_GRP)[None, :],
                       (BLK, 1)).astype(BF16)
    ident = np.eye(BLK, dtype=np.float32).astype(BF16)

    in_maps = []
    for c in range(C):
        pc = per_core[c]
        in_maps.append({
            "xTf": xTf,
            "xTown": np.ascontiguousarray(xTf[:, c * NP:(c + 1) * NP]),
            "distT": pc["distT"],
            "idxw": pc["idxw"],
            "dlw": pc["dlw"],
            "iota": iota_rep,
            "ident": ident,
            "W1": np.asarray(W1, np.float32).astype(BF16),
            "W2": np.asarray(W2, np.float32).astype(BF16),
            "b1": np.asarray(b1, np.float32).reshape(HID, 1),
            "b2": np.asarray(b2, np.float32).reshape(OUT, 1),
        })
    return in_maps


def _dense_chunks(nblocks, ch):
    out = []
    left = nblocks
    while left > 0:
        out.append(min(ch, left))
        left -= out[-1]
    return out


# ----------------------------------------------------------------------------
# Device program (identical on all 8 cores)
# ----------------------------------------------------------------------------

def _build(meta):
    from concourse import bacc, mybir, tile

    sweeps = meta["sweeps"]
    nsw = len(sweeps)
    sweep_base = meta["sweep_base"]
    g_sj = meta["g_sj"]
    call_base = meta["call_base"]
    tiles_total = meta["tiles_total"]
    sched = meta["sched"]
    id_flags = meta["id_flags"]
    qblk0 = [int(b) // BLK for b in QBASE]
    f32 = mybir.dt.float32
    bf16 = mybir.dt.bfloat16

    nc = bacc.Bacc(num_devices=C)
    d_xTf = nc.dram_tensor("xTf", [IN, C * NP], bf16, kind="ExternalInput")
    d_xTown = nc.dram_tensor("xTown", [IN, NP], bf16, kind="ExternalInput")
    d_distT = nc.dram_tensor("distT", [64, NP], bf16, kind="ExternalInput")
    d_idxw = nc.dram_tensor("idxw", [128, meta["slots_total"] // 16],
                            mybir.dt.int16, kind="ExternalInput")
    d_dlw = nc.dram_tensor("dlw", [128, tiles_total], bf16, kind="ExternalInput")
    d_iota = nc.dram_tensor("iota", [BLK, BLK * OH_GRP], bf16, kind="ExternalInput")
    d_ident = nc.dram_tensor("ident", [BLK, BLK], bf16, kind="ExternalInput")
    d_W1 = nc.dram_tensor("W1", [IN, HID], bf16, kind="ExternalInput")
    d_W2 = nc.dram_tensor("W2", [HID, OUT], bf16, kind="ExternalInput")
    d_b1 = nc.dram_tensor("b1", [HID, 1], f32, kind="ExternalInput")
    d_b2 = nc.dram_tensor("b2", [OUT, 1], f32, kind="ExternalInput")
    d_out = nc.dram_tensor("outT", [OUT, NP], f32, kind="ExternalOutput")

    gmax = int(g_sj.max())

    with tile.TileContext(nc) as tc:
        with (
            tc.tile_pool(name="persist", bufs=1) as pp,
            tc.tile_pool(name="dram", bufs=1, space="DRAM") as dp,
        ):
            t_dlw = pp.tile([128, tiles_total], bf16, tag="dlw")
            t_iota = pp.tile([BLK, BLK * OH_GRP], bf16, tag="iota")
            t_ident = pp.tile([BLK, BLK], bf16, tag="ident")
            t_W1 = pp.tile([IN, HID], bf16, tag="W1")
            t_W2 = pp.tile([HID, OUT], bf16, tag="W2")
            t_b1 = pp.tile([HID, 1], f32, tag="b1")
            t_b2 = pp.tile([OUT, 1], f32, tag="b2")
            t_distT = pp.tile([64, NP], bf16, tag="distT")
            t_h1T = pp.tile([64, NP], bf16, tag="h1T")
            t_g1own = pp.tile([128, NBLK * 64], bf16, tag="g1own")
            t_g2own = pp.tile([128, NBLK * OUT], bf16, tag="g2own")
            t_idxw = pp.tile([128, meta["slots_total"] // 16], mybir.dt.int16,
                             tag="idxw")
            # W1/b1 are needed by the dense phase immediately; everything
            # else is edge-phase-only and is loaded during phase A's compute
            # window (see the j == 1 hook below) so the first table writes
            # start as early as possible.
            nc.sync.dma_start(out=t_W1[:], in_=d_W1[:])
            nc.sync.dma_start(out=t_b1[:], in_=d_b1[:])

            tab1 = [dp.tile([TROWS[j], BLK], bf16, name=f"tab1_{j}",
                            tag=f"tab1_{j}") for j in range(4)]
            tab2 = [dp.tile([TROWS[j], BLK], bf16, name=f"tab2_{j}",
                            tag=f"tab2_{j}") for j in range(4)]
            own2c = dp.tile([NP, OUT], bf16, name="own2c", tag="own2c")
            cc2 = [dp.tile([TROWS[j], OUT], bf16, name=f"cc2_{j}",
                           tag=f"cc2_{j}") for j in range(4)]

            # ---- phase A: replicated dense L1 -> DRAM tables (chunk-major),
            # with the own-shard dense (-> t_g1own) interleaved after chunk 0
            # so its compute overlaps chunk-1 loads.
            def own_dense(xp0, qp0d):
                t_xo = xp0.tile([IN, NP], bf16, tag="xo")
                nc.sync.dma_start(out=t_xo[:], in_=d_xTown[:])
                bb = 0
                for nb in _dense_chunks(NBLK, DCH):
                    p = qp0d.tile([128, DCH * 64], f32, tag="p0")
                    for t in range(nb):
                        nc.tensor.matmul(
                            out=p[:, t * 64:(t + 1) * 64],
                            lhsT=t_xo[:, (bb + t) * BLK:(bb + t + 1) * BLK],
                            rhs=t_W1[:],
                            start=(t == 0), stop=(t == nb - 1),
                        )
                    nc.scalar.activation(
                        out=t_g1own[:, bb * 64:(bb + nb) * 64],
                        in_=p[:, :nb * 64],
                        func=mybir.ActivationFunctionType.Copy,
                    )
                    bb += nb

            with (
                tc.tile_pool(name="dz0x", bufs=1) as xp0,
                tc.tile_pool(name="dz0p", bufs=3, space="PSUM") as qp0d,
                tc.tile_pool(name="dz1s", bufs=4) as sp1,
                tc.tile_pool(name="dz1x", bufs=4) as xp1,
                tc.tile_pool(name="dz1p", bufs=4, space="PSUM") as qp1d,
            ):
                def load_xs(j, r):
                    xs = xp1.tile([IN, max(QROWS)], bf16, tag="xs")
                    nc.sync.dma_start(
                        out=xs[:, :QROWS[j]],
                        in_=d_xTf[:, r * NP + int(QBASE[j]):
                                  r * NP + int(QBASE[j]) + QROWS[j]],
                    )
                    return xs

                pairs = [(j, r) for j in range(4) for r in range(C)]
                xsq = [load_xs(*pairs[0]), load_xs(*pairs[1])]
                for pi, (j, r) in enumerate(pairs):
                    xs = xsq.pop(0)
                    if pi + 2 < len(pairs):
                        # keep two loads in flight past this pair's table
                        # write so its eviction wait never stalls the loads
                        xsq.append(load_xs(*pairs[pi + 2]))
                    if pi == C:  # chunk 0 done
                        own_dense(xp0, qp0d)
                        nc.sync.dma_start(out=t_idxw[:], in_=d_idxw[:])
                        nc.sync.dma_start(out=t_dlw[:], in_=d_dlw[:])
                        nc.sync.dma_start(out=t_iota[:], in_=d_iota[:])
                        nc.sync.dma_start(out=t_ident[:], in_=d_ident[:])
                        nc.sync.dma_start(out=t_W2[:], in_=d_W2[:])
                        nc.sync.dma_start(out=t_b2[:], in_=d_b2[:])
                        nc.sync.dma_start(out=t_distT[:], in_=d_distT[:])
                    tabv = tab1[j][:].rearrange("(t p) f -> p t f", p=128)
                    ev = sp1.tile([128, QB[j] * 64], bf16, tag="ev")
                    bb = 0
                    ci = 0
                    for nb in _dense_chunks(QB[j], DCH):
                        p = qp1d.tile([128, DCH * 64], f32, tag="p1")
                        for t in range(nb):
                            nc.tensor.matmul(
                                out=p[:, t * 64:(t + 1) * 64],
                                lhsT=xs[:, (bb + t) * BLK:(bb + t + 1) * BLK],
                                rhs=t_W1[:],
                                start=(t == 0), stop=(t == nb - 1),
                            )
                        if ci % 2 == 0:
                            nc.scalar.activation(
                                out=ev[:, bb * 64:(bb + nb) * 64],
                                in_=p[:, :nb * 64],
                                func=mybir.ActivationFunctionType.Copy,
                            )
                        else:
                            nc.vector.tensor_scalar_mul(
                                ev[:, bb * 64:(bb + nb) * 64],
                                p[:, :nb * 64], 1.0,
                            )
                        bb += nb
                        ci += 1
                    nc.sync.dma_start(
                        out=tabv[:, r * QB[j]:(r + 1) * QB[j], :64],
                        in_=ev[:].rearrange("p (t f) -> p t f", f=64),
                    )

            # ---- edge sweep (shared by both layers), split into parts so
            # phase C can defer chunk-3 work past the last collective
            def sweep_open(L, s, qp):
                nb, _q = sweeps[s]
                nf = 64 if L == 1 else OUT
                gown = t_g1own if L == 1 else t_g2own
                ps = qp.tile([nf, SWMAX * BLK], f32, tag="ps")
                for lb in range(nb):
                    blk = int(sweep_base[s]) + lb
                    fst, lst = id_flags[s][lb]
                    nc.tensor.matmul(
                        out=ps[:, lb * BLK:(lb + 1) * BLK],
                        lhsT=gown[:, blk * nf:(blk + 1) * nf],
                        rhs=t_ident[:],
                        start=fst, stop=lst,
                    )
                return ps

            def chunk_gather(L, s, j, gp):
                tabs = tab1 if L == 1 else tab2
                G = int(g_sj[s, j])
                if G == 0:
                    return None
                tb = int(call_base[s, j])
                gb = gp.tile([128, gmax, BLK], bf16, tag="gb")
                nc.gpsimd.dma_gather(
                    out_ap=gb[:, :G, :],
                    in_ap=tabs[j][:, :],
                    idxs_ap=t_idxw[:, tb * 8:tb * 8 + G * 8],
                    num_idxs=G * BLK,
                    num_idxs_reg=G * BLK,
                    elem_size=BLK,
                    single_packet=False,
                )
                return gb

            def sweep_chunk(L, s, ps, j, gp, op_, gb=None):
                nf = 64 if L == 1 else OUT
                G = int(g_sj[s, j])
                if G == 0:
                    return
                tb = int(call_base[s, j])
                if gb is None:
                    gb = chunk_gather(L, s, j, gp)
                todo = sched[s][j]
                for g0 in range(0, len(todo), OH_GRP):
                    grp = todo[g0:g0 + OH_GRP]
                    ng = len(grp)
                    oh = op_.tile([128, BLK, OH_GRP], bf16, tag="oh")
                    c0 = tb + grp[0][0]
                    nc.vector.tensor_tensor(
                        out=oh[:, :, :ng],
                        in0=t_iota[:].rearrange(
                            "p (j k) -> p j k", k=OH_GRP)[:, :, :ng],
                        in1=t_dlw[:, c0:c0 + ng].unsqueeze(1)
                            .to_broadcast([128, BLK, ng]),
                        op=mybir.AluOpType.is_equal,
                    )
                    for k, (cu, lb, stp) in enumerate(grp):
                        nc.tensor.matmul(
                            out=ps[:, lb * BLK:(lb + 1) * BLK],
                            lhsT=gb[:, cu, :nf],
                            rhs=oh[:, :, k],
                            start=False, stop=stp,
                        )

            def sweep_fin(L, s, ps, fp, sop, split_out=False):
                nb, _q = sweeps[s]
                nf = 64 if L == 1 else OUT
                bias = t_b1 if L == 1 else t_b2
                if L == 2:
                    ob = sop.tile([OUT, SWMAX * BLK], f32, tag="ob")
                for lb in range(nb):
                    gcol = (int(sweep_base[s]) + lb) * BLK
                    ft = fp.tile([nf, BLK], f32, tag="ft")
                    nc.vector.tensor_tensor(
                        out=ft[:],
                        in0=ps[:, lb * BLK:(lb + 1) * BLK],
                        in1=t_distT[:nf, gcol:gcol + BLK],
                        op=mybir.AluOpType.mult,
                    )
                    if L == 1:
                        fa = fp.tile([nf, BLK], f32, tag="fa")
                        nc.scalar.activation(
                            out=fa[:], in_=ft[:],
                            func=mybir.ActivationFunctionType.Relu,
                            bias=bias[:, :1], scale=1.0,
                        )
                        nc.vector.tensor_tensor(
                            out=t_h1T[:, gcol:gcol + BLK],
                            in0=fa[:],
                            in1=t_distT[:, gcol:gcol + BLK],
                            op=mybir.AluOpType.mult,
                        )
                    else:
                        nc.scalar.activation(
                            out=ob[:, lb * BLK:(lb + 1) * BLK], in_=ft[:],
                            func=mybir.ActivationFunctionType.Relu,
                            bias=bias[:, :1], scale=1.0,
                        )
                        if split_out:
                            nc.sync.dma_start(
                                out=d_out[:, gcol:gcol + BLK],
                                in_=ob[:, lb * BLK:(lb + 1) * BLK],
                            )
                if L == 2 and not split_out:
                    c0 = int(sweep_base[s]) * BLK
                    nc.sync.dma_start(
                        out=d_out[:, c0:c0 + nb * BLK],
                        in_=ob[:, :nb * BLK],
                    )

            def edge_sweep(L, s, gp, op_, fp, qp, sop, split_out=False):
                ps = sweep_open(L, s, qp)
                for j in range(4):
                    sweep_chunk(L, s, ps, j, gp, op_)
                sweep_fin(L, s, ps, fp, sop, split_out)

            own2v = own2c[:].rearrange("(t p) f -> p t f", p=128)
            # ---- phase B: L1 edge + per-quarter L2 dense + CC + expand
            with (
                tc.tile_pool(name="eg0", bufs=5) as gp0,
                tc.tile_pool(name="eo0", bufs=3) as op0,
                tc.tile_pool(name="ef0", bufs=4) as fp0,
                tc.tile_pool(name="ep0", bufs=2, space="PSUM") as qp0,
                tc.tile_pool(name="es0", bufs=2) as sop0,
                tc.tile_pool(name="dz2p", bufs=2, space="PSUM") as qp2,
            ):
                def dense2(b0, nblocks):
                    bb = 0
                    for nb in _dense_chunks(nblocks, DCH):
                        bglob = b0 + bb
                        p2 = qp2.tile([128, DCH * OUT], f32, tag="p2")
                        for t in range(nb):
                            nc.tensor.matmul(
                                out=p2[:, t * OUT:(t + 1) * OUT],
                                lhsT=t_h1T[:, (bglob + t) * BLK:
                                           (bglob + t + 1) * BLK],
                                rhs=t_W2[:],
                                start=(t == 0), stop=(t == nb - 1),
                            )
                        nc.scalar.activation(
                            out=t_g2own[:, bglob * OUT:(bglob + nb) * OUT],
                            in_=p2[:, :nb * OUT],
                            func=mybir.ActivationFunctionType.Copy,
                        )
                        nc.sync.dma_start(
                            out=own2v[:, bglob:bglob + nb, :],
                            in_=t_g2own[:, bglob * OUT:(bglob + nb) * OUT]
                                .rearrange("p (t f) -> p t f", f=OUT),
                        )
                        bb += nb

                for qq in range(4):
                    # interleave: L2 dense for sweep s-1's blocks right after
                    # sweep s starts consuming the queues
                    prev = None
                    for s in range(nsw):
                        if sweeps[s][1] == qq:
                            edge_sweep(1, s, gp0, op0, fp0, qp0, sop0)
                            if prev is not None:
                                dense2(int(sweep_base[prev]), sweeps[prev][0])
                            prev = s
                    dense2(int(sweep_base[prev]), sweeps[prev][0])
                    nc.gpsimd.collective_compute(
                        "AllGather", mybir.AluOpType.bypass,
                        replica_groups=[list(range(C))],
                        ins=[own2c[int(QBASE[qq]):int(QBASE[qq]) + QROWS[qq],
                                   :].opt()],
                        outs=[cc2[qq][:].opt()],
                    )
                    # expand compact [rows, 32] into 256B-stride table rows
                    nc.sync.dma_start(
                        out=tab2[qq][:, :OUT],
                        in_=cc2[qq][:, :],
                    )

            # ---- phase C: L2 edge. The first two sweeps emit chunks 0-2
            # for both sweeps before either touches chunk 3, so the Pool/DMA
            # queues stay fed while the final AllGather + expand complete.
            with (
                tc.tile_pool(name="eg1", bufs=9) as gp1,
                tc.tile_pool(name="eo1", bufs=3) as op1,
                tc.tile_pool(name="ef1", bufs=4) as fp1,
                tc.tile_pool(name="ep1", bufs=2, space="PSUM") as qp1,
                tc.tile_pool(name="es1", bufs=2) as sop1,
            ):
                ndef = 2
                pss = []
                for s in range(ndef):
                    ps_ = sweep_open(2, s, qp1)
                    for j in range(3):
                        sweep_chunk(2, s, ps_, j, gp1, op1)
                    pss.append(ps_)
                # prefetch sweep-2 chunk gathers into spare gb buffers while
                # the last collective finishes (gathers need no PSUM)
                pre2 = [chunk_gather(2, ndef, j, gp1) for j in range(3)]
                for s in range(ndef):
                    sweep_chunk(2, s, pss[s], 3, gp1, op1)
                    sweep_fin(2, s, pss[s], fp1, sop1)
                ps_c = sweep_open(2, ndef, qp1)
                for j in range(3):
                    sweep_chunk(2, ndef, ps_c, j, gp1, op1, gb=pre2[j])
                sweep_chunk(2, ndef, ps_c, 3, gp1, op1)
                sweep_fin(2, ndef, ps_c, fp1, sop1)
                for s in range(ndef + 1, nsw):
                    edge_sweep(2, s, gp1, op1, fp1, qp1, sop1,
                               split_out=(s == nsw - 1))

    nc.finalize()
    return nc


# ----------------------------------------------------------------------------
# Entry point
# ----------------------------------------------------------------------------

_CACHE = {}


def _prepare(x, edge_index, W1, b1, W2, b2):
    ei = np.asarray(edge_index, dtype=np.int64)
    key = (ei.shape, hash(ei[:, ::65537].tobytes()))
    if _CACHE.get("key") != key:
        meta, per_core, dis, node_core, node_off = _pack(ei)
        nc = _build(meta)
        _CACHE.update(key=key, meta=meta, per_core=per_core, nc=nc,
                      dis=dis, node_core=node_core, node_off=node_off)
    in_maps = _stage_inputs(x, W1, b1, W2, b2, _CACHE["meta"],
                            _CACHE["per_core"], _CACHE["dis"],
                            _CACHE["node_core"], _CACHE["node_off"])
    return _CACHE["nc"], in_maps


def kernel(x, edge_index, W1, b1, W2, b2):
    from concourse.bass_utils import run_bass_kernel_spmd

    nc, in_maps = _prepare(x, edge_index, W1, b1, W2, b2)
    res = run_bass_kernel_spmd(nc, in_maps, core_ids=list(range(C)))
    node_core = _CACHE["node_core"]
    node_off = _CACHE["node_off"]
    out = np.zeros((N, OUT), np.float32)
    for c in range(C):
        ids = np.where(node_core == c)[0]
        out[ids] = np.asarray(res.results[c]["outT"], np.float32)[:, node_off[ids]].T
    return out


# ----------------------------------------------------------------------------
# Host-side emulation (validates packing + schedule; no HW)
# ----------------------------------------------------------------------------

def emulate(x, edge_index, W1, b1, W2, b2):
    x = np.asarray(x, np.float32)
    ei = np.asarray(edge_index, np.int64)
    meta, per_core, dis, node_core, node_off = _pack(ei)
    sweeps, sweep_base = meta["sweeps"], meta["sweep_base"]
    g_sj, call_base = meta["g_sj"], meta["call_base"]
    sched = meta["sched"]
    W1 = np.asarray(W1, np.float32).astype(BF16).astype(np.float32)
    W2 = np.asarray(W2, np.float32).astype(BF16).astype(np.float32)
    b1 = np.asarray(b1, np.float32)
    b2 = np.asarray(b2, np.float32)

    xp = (x * dis[:, None]).astype(BF16).astype(np.float32)
    col = node_core.astype(np.int64) * NP + node_off
    xTf = np.zeros((C * NP, IN), np.float32)
    xTf[col] = xp

    def run_layer(actsT, W, bias, nf):
        """actsT: [C*NP, 64-or-?] padded per-rank activations (already *dis).
        Returns per-core scatter result after finalize (pre-next-scale)."""
        g = (actsT @ W).astype(BF16).astype(np.float32)  # [C*NP, nf]
        gtabs = []
        for j in range(4):
            rows = []
            for r in range(C):
                a = r * NP + int(QBASE[j])
                rows.append(g[a:a + QROWS[j]])
            gtabs.append(np.concatenate(rows))
        outs = []
        for c in range(C):
            pc = per_core[c]
            idxw, dlw = pc["idxw"], pc["dlw"].astype(np.float32)
            disr = pc["distT"][0].astype(np.float32)
            sT = np.zeros((nf, NP), np.float32)
            # identity (self-loop) contribution
            gown = g[c * NP:(c + 1) * NP, :nf]
            sT += gown.T
            for s in range(len(sweeps)):
                for j in range(4):
                    G = int(g_sj[s, j])
                    if G == 0:
                        continue
                    tb = int(call_base[s, j])
                    iw = idxw[:16, tb * 8:(tb + G) * 8]
                    idxs = iw.T.reshape(-1)
                    rows = gtabs[j][idxs]
                    for (cu, lb, _st) in sched[s][j]:
                        t = tb + cu
                        msg = rows[cu * BLK:(cu + 1) * BLK, :nf]
                        dl = dlw[:, t]
                        oh = (dl[:, None] ==
                              np.arange(BLK, dtype=np.float32)[None, :])
                        bcol = (int(sweep_base[s]) + lb) * BLK
                        sT[:, bcol:bcol + BLK] += msg.T @ oh
                    # (tiles are ordered by block within the call)
            act = np.maximum(sT * disr[None, :] + bias[:nf].reshape(-1, 1), 0.0)
            outs.append(act)
        return outs

    h1 = run_layer(xTf, W1, b1, 64)
    h1p = []
    for c in range(C):
        disr = per_core[c]["distT"][0].astype(np.float32)
        h = (h1[c] * disr[None, :]).astype(BF16).astype(np.float32)
        a = np.zeros((NP, 64), np.float32)
        a[:, :64] = h.T
        h1p.append(a)
    h1all = np.concatenate(h1p)
    out2 = run_layer(h1all, W2, b2, OUT)

    out = np.zeros((N, OUT), np.float32)
    for c in range(C):
        ids = np.where(node_core == c)[0]
        out[ids] = out2[c][:, node_off[ids]].T
    return out
